# revision 50
# baseline (speedup 1.0000x reference)
"""Trainium2 Bass kernel for nn_CountingDiceLoss.

Reference math (B=8, H=W=512, P=40 centroids, 2-class dice + density-map MSE
+ squared count error):

  dm   = (sum_p exp(-((i-ci_p)^2+(j-cj_p)^2)/(2 s_k^2)) / (srpi*s_k))
         * bbox_mask / 2.50635
  p1   = softmax(x[:, :2])[:, 1] == sigmoid(x1 - x0)
  dc   = (2 tp + s) / (sum p1 + sum y + s)      (tp/fp/fn algebraic identity)
  loss = -mean_b(dc) + mean((x2 - dm)^2) + (sum x2 - sum dm)^2

Structure exploited:
  * The gaussian is separable: exp(-(di^2+dj^2)/2) = exp(-di^2/2)*exp(-dj^2/2),
    so the P-component accumulation is a rank-P outer-product sum — a
    [H,P] @ [P,W] TensorEngine matmul. The tiny 1-D factor tables
    (B*P*(H+W) elements, 0.3% of the input bytes) are precomputed on host
    with np.exp (also matches the reference's CPU f32 exp better than the
    ACT table, which has a ~1e-5 systematic bias).
  * Every reduction is fused into an elementwise pass it already needed
    (activation / scalar_tensor_tensor accum_out), finished in f64 on host.
    sum(x2) comes free via the identity sum(x2) = sum(x2-dm) + sum(dm);
    sum(y) is exact integer column sums via PE ones-matmuls after the
    density-map matmuls retire.
  * One ~0.5-1MB dma_start per map piece with 8KB-contiguous runs (4 rows
    per partition) reaches HBM line rate; all DMAs share one FIFO HWDGE
    ring, so issue order = arrival order, chosen so each input's dependent
    chain overlaps the remaining stream (y and x2 are split in halves to
    pipeline the dm-mask and err->square tails).
  * Mixed precision: x0/x1/y/mask stream as bf16 (half the bytes, 2x DVE
    on the subtract). These feed only the dice term, ~1e-7 of the loss
    (error budget ~1e-6 rel even if l_n vanished); y/mask are 0/1 so the
    mask-multiply and sum(y) stay EXACT. x2 and the gaussian tables stay
    f32 — they feed l_n, the dominant term.
  * Per-q PSUM tiles make each PE->DVE handoff per-matmul (dependency
    tracking is tile-granular — one psum tile would stall the mask
    multiply until ALL matmuls retire); an order-only add_dep_helper pins
    the tp pass after the err chain so the scheduler cannot hoist it into
    the critical path; a dummy early activation hoists the ACT
    function-table load off the first sigmoid.
  * When bbox_mask == y (true for the reference generator), one load is
    dropped and the y tile doubles as the mask (separate-variant fallback
    compiled on demand).

Sharding: data-parallel over batch; core c handles sample b=c (B == 8 cores).
"""

import numpy as np

import concourse.bacc as bacc
import concourse.bass as bass  # noqa: F401  (kept for users of this module)
import concourse.mybir as mybir
import concourse.tile as tile
from concourse.bass_utils import run_bass_kernel_spmd

B, H, W, P = 8, 512, 512, 40
NCORES = 8
RT = 128                 # partition tile
Q = H // RT              # 4 rows per partition (8KB contiguous DMA runs)
NSTAT = 12               # p1_ab, dm_ab, tp_ab, sqerr_abc, err_abc

_sk = 2.0 ** (1.0 / 1e11)
_srpi = float(np.sqrt(2.0 * np.pi))
EXP_SCALE = float(-1.0 / (2.0 * _sk * _sk))      # ~ -0.5
POST = float(1.0 / (_srpi * _sk) / 2.50635)      # folded normalization

_F32 = mybir.dt.float32
_BF16 = mybir.dt.bfloat16


def _emit(tc, nc, xc, x2c, yc, mc, g_d, stats_out, sy_out, shared_mask):
    A = mybir.AluOpType
    AF = mybir.ActivationFunctionType

    with (
        tc.tile_pool(name="const", bufs=1) as cpool,
        tc.tile_pool(name="inp", bufs=1) as ipool,
        tc.tile_pool(name="scr", bufs=1) as spool,
        tc.tile_pool(name="stat", bufs=1) as stpool,
        tc.tile_pool(name="psum", bufs=1, space="PSUM") as ppool,
    ):
        # ---- input DMAs, one FIFO HWDGE ring (issue order == arrival
        # order). The dice-only inputs (x0, x1, y, mask) arrive as bf16 —
        # the dice term is ~1e-7 of the loss, so bf16 is invisible there —
        # halving their HBM bytes; x2 and the gaussian tables stay f32
        # because they feed l_n, the dominant loss term.
        HQ = Q // 2

        def map_tile(ap, tag, dt=_F32):
            t = ipool.tile([RT, Q, W], dt, tag=tag)
            return t, ap.rearrange("(p q) j -> p q j", p=RT)

        def load(t, src, a, b):
            nc.sync.dma_start(t[:, a:b], src[:, a:b])

        x0t, x0src = map_tile(xc[0], "x0t", _BF16)
        x1t, x1src = map_tile(xc[1], "x1t", _BF16)
        x2t, x2src = map_tile(x2c[:], "x2t")
        yt, ysrc = map_tile(yc[:], "yt", _BF16)
        gt = cpool.tile([P, 2, H], _F32)
        nc.sync.dma_start(gt[:], g_d[:])
        gi, gj = gt[:, 0, :], gt[:, 1, :]
        load(x0t, x0src, 0, Q)
        load(x1t, x1src, 0, Q)
        if shared_mask:
            mt = yt
            load(yt, ysrc, 0, HQ)
            load(yt, ysrc, HQ, Q)
        else:
            mt, msrc = map_tile(mc[:], "mt", _BF16)
            load(mt, msrc, 0, Q)
            load(yt, ysrc, 0, Q)
        load(x2t, x2src, 0, HQ)
        load(x2t, x2src, HQ, Q)

        stats_sb = stpool.tile([RT, NSTAT], _F32)
        nc.gpsimd.memset(stats_sb[:], 0.0)
        # one psum tile per q so each PE->DVE handoff is per-matmul (a
        # single psum tile would make the mask-multiply wait for ALL
        # matmuls: dependency tracking is tile-granular)
        dmp = [
            ppool.tile([RT, W], _F32, tag=f"dmp{q}", name=f"dmp{q}")
            for q in range(Q)
        ]

        def col(s):
            return stats_sb[:, s:s + 1]

        # tiny dummy activation so the ACT function-table load runs while
        # ACT is idle instead of attached to the first real sigmoid
        dummy = stpool.tile([1, 1], _F32)
        nc.gpsimd.memset(dummy[:], 0.0)
        nc.scalar.activation(dummy[:], dummy[:], AF.Sigmoid)

        # density map rows: partition p, free (q, j) holds row 4p+q
        gi_q = gi.rearrange("a (p q) -> a p q", q=Q)
        for q in range(Q):
            nc.tensor.matmul(
                dmp[q][:], gi_q[:, :, q], gj[:], start=True, stop=True,
            )

        # sum(y): exact integer column sums via PE ones-matmul (PE is idle
        # once the 4 density-map matmuls finish)
        ones = cpool.tile([RT, 1], _BF16)
        nc.gpsimd.memset(ones[:], 1.0)
        sy_ps = ppool.tile([1, W], _F32, tag="sy_ps")
        for q in range(Q):
            nc.tensor.matmul(
                sy_ps[:], ones[:, 0:1], yt[:, q, :],
                start=q == 0, stop=q == Q - 1, skip_group_check=True,
            )
        sy_sb = stpool.tile([1, W], _F32)
        nc.scalar.copy(sy_sb[:], sy_ps[:])

        halves = [(0, HQ), (HQ, Q)]

        # p1 = sigmoid(x1 - x0); accum sum(p1) in f32 (bf16 data path)
        t01 = spool.tile([RT, Q, W], _BF16)
        p1 = spool.tile([RT, Q, W], _BF16)
        nc.vector.tensor_sub(t01[:], x1t[:], x0t[:])
        nc.scalar.activation(p1[:], t01[:], AF.Sigmoid, accum_out=col(0))

        # dm = (psum_q * POST) * mask_q per q (starts on each matmul's
        # completion); err = x2 - dm per half with accum sum(err)
        # [sum(x2) = sum(err) + sum(dm)]; squares on ACT as halves finish.
        dmm = spool.tile([RT, Q, W], _F32)
        err = spool.tile([RT, Q, W], _F32)

        def dmm_q(q):
            nc.vector.scalar_tensor_tensor(
                dmm[:, q, :], dmp[q][:], POST, mt[:, q, :],
                op0=A.mult, op1=A.mult, accum_out=col(2 + q),
            )

        def err_h(h, a, b):
            e = nc.vector.scalar_tensor_tensor(
                err[:, a:b], x2t[:, a:b], 1.0, dmm[:, a:b],
                op0=A.mult, op1=A.subtract, accum_out=col(8 + h),
            )
            sq = spool.tile([RT, b - a, W], _F32, tag=f"sq{h}")
            nc.scalar.activation(
                sq[:], err[:, a:b], AF.Square, accum_out=col(6 + h),
            )
            return e

        dmm_q(0)
        dmm_q(1)
        err_h(0, 0, HQ)
        dmm_q(2)
        dmm_q(3)
        last_err = err_h(1, HQ, Q)

        # tp partial: sum(p1 * y), bf16 inputs with f32 accumulator. Pin it
        # after the final err op (order-only dep): its inputs are ready
        # early and the scheduler would otherwise hoist it into the
        # err/dm critical chain.
        prod = spool.tile([RT, Q, W], _BF16)
        prod_i = nc.vector.scalar_tensor_tensor(
            prod[:], p1[:], 1.0, yt[:], op0=A.mult, op1=A.mult,
            accum_out=col(1),
        )
        tile.add_dep_helper(
            prod_i.ins, last_err.ins, sync=False,
            reason="keep tp off the err critical chain",
        )

        nc.sync.dma_start(stats_out[:], stats_sb[:])
        nc.sync.dma_start(sy_out[:], sy_sb[:])


_BUILT = {}


def _build(shared_mask):
    if shared_mask not in _BUILT:
        nc = bacc.Bacc(
            "TRN2", target_bir_lowering=False, debug=False, num_devices=NCORES,
        )
        xc = nc.dram_tensor(
            "x01", [2, H, W], _BF16, kind="ExternalInput"
        ).ap()
        x2c = nc.dram_tensor("x2", [H, W], _F32, kind="ExternalInput").ap()
        yc = nc.dram_tensor("yc", [H, W], _BF16, kind="ExternalInput").ap()
        mc = None
        if not shared_mask:
            mc = nc.dram_tensor(
                "mc", [H, W], _BF16, kind="ExternalInput"
            ).ap()
        g_d = nc.dram_tensor("g", [P, 2, H], _F32, kind="ExternalInput").ap()
        stats = nc.dram_tensor(
            "stats", [RT, NSTAT], _F32, kind="ExternalOutput"
        ).ap()
        sy = nc.dram_tensor("sy", [1, W], _F32, kind="ExternalOutput").ap()
        with tile.TileContext(nc) as tc:
            _emit(tc, nc, xc, x2c, yc, mc, g_d, stats, sy, shared_mask)
        nc.compile()
        _BUILT[shared_mask] = nc
    return _BUILT[shared_mask]


def make_in_maps(x, y, bbox_mask, centroids, valid, shared_mask):
    import ml_dtypes

    bf16 = ml_dtypes.bfloat16
    x = np.asarray(x, dtype=np.float32)
    x01 = np.ascontiguousarray(x[:, :2].astype(bf16))
    x2 = np.ascontiguousarray(x[:, 2])
    y = np.ascontiguousarray(np.asarray(y, dtype=np.float32).astype(bf16))
    bbox_mask = np.ascontiguousarray(
        np.asarray(bbox_mask, dtype=np.float32).astype(bf16)
    )
    centroids = np.asarray(centroids)
    validf = np.asarray(valid).astype(np.float32)

    # 1-D gaussian factor tables (separable kernel), f32 like the reference
    idx = np.arange(H, dtype=np.float32)
    ci = centroids[..., 0].astype(np.float32)[..., None]   # [B,P,1]
    cj = centroids[..., 1].astype(np.float32)[..., None]
    gi = np.exp(((idx[None, None, :] - ci) ** 2) * np.float32(EXP_SCALE))
    gi = gi * validf[..., None]
    gj = np.exp(((idx[None, None, :] - cj) ** 2) * np.float32(EXP_SCALE))
    g = np.ascontiguousarray(np.stack([gi, gj], axis=2).astype(np.float32))

    maps = []
    for c in range(NCORES):
        m = {"x01": x01[c], "x2": x2[c], "yc": y[c, 0], "g": g[c]}
        if not shared_mask:
            m["mc"] = bbox_mask[c, 0]
        maps.append(m)
    return maps


def combine(results):
    """results: per-core dicts with stats [128, NSTAT] -> scalar loss."""
    s = np.stack(
        [r["stats"].astype(np.float64).sum(axis=0) for r in results]
    )  # [B, NSTAT]
    sum_p1 = s[:, 0]
    tp = s[:, 1]
    sum_dm = s[:, 2:6].sum(axis=1)
    sum_sq = s[:, 6] + s[:, 7]
    sum_x2 = s[:, 8] + s[:, 9] + sum_dm
    sum_y = np.array(
        [r["sy"].astype(np.float64).sum() for r in results]
    )
    smooth = 1e-5
    dc = (2.0 * tp + smooth) / (sum_p1 + sum_y + smooth)
    l_dice = -dc.mean()
    l_dm = sum_sq.sum() / (B * H * W)
    l_n = (sum_x2.sum() - sum_dm.sum()) ** 2
    return np.float32(l_dice + l_dm + l_n)


LAST_RESULT = None  # BassKernelResults of the most recent run (for profiling)


def kernel(x, y, bbox_mask, centroids, valid):
    global LAST_RESULT
    shared = np.array_equal(
        np.asarray(y, dtype=np.float32), np.asarray(bbox_mask, dtype=np.float32)
    )
    nc = _build(shared)
    in_maps = make_in_maps(x, y, bbox_mask, centroids, valid, shared)
    res = run_bass_kernel_spmd(nc, in_maps, list(range(NCORES)))
    LAST_RESULT = res
    return combine(res.results)
